# revision 41
# baseline (speedup 1.0000x reference)
"""Trainium2 Bass kernel: Swin-style window attention with relative position bias.

Self-contained: hardcodes B=64, N=576, C=768, H=12. Shards batch over 8 cores.

Per-core design (fully on-device compute; host only reshapes/sharding):
 - qk projection emitted in transposed layout qkT[feat, tok] (attention scale
   folded into q weights), v in normal layout [tok, feat] padded with a ones
   column per head (softmax denominator trick).
 - S_T[j,i] = k^T q computed per (batch, head); softmax without max-subtract
   (logits are bounded): P = exp(S_T) * exp(bias_T), bias exp-table built on
   host from rpb_table[rel_idx] (works for arbitrary rel_idx).
 - PV: oD[65, i] = [v | 1]^T P_T accumulated over j-chunks -> row 64 holds the
   softmax denominator. Reciprocal batched per batch, broadcast via stride-0
   DMA, multiply, assemble oT, project with bias via ones-row.
"""
import sys

sys.path.insert(0, "/opt/trn_rl_repo")

import numpy as np
import ml_dtypes

BF16 = ml_dtypes.bfloat16

B, N, C = 64, 576, 768
H, D = 12, 64
NCORES = 8
BL = B // NCORES           # 8 batches per core
NTOK = BL * N              # 4608 tokens per core

# token/j tiles of N=576: 4x128 + 1x64
JT = [(0, 128), (128, 128), (256, 128), (384, 128), (512, 64)]

_cache = {}


def _build(reps=1):
    key = ("nc", reps)
    if key in _cache:
        return _cache[key]
    from contextlib import ExitStack
    import concourse.tile as tile
    from concourse import bacc, mybir

    f32 = mybir.dt.float32
    bf16 = mybir.dt.bfloat16
    EXP = mybir.ActivationFunctionType.Exp

    nc = bacc.Bacc("TRN2", target_bir_lowering=False, debug=False,
                   num_devices=NCORES)
    xT = nc.dram_tensor("xT", [C, NTOK], bf16, kind="ExternalInput").ap()
    wqk = nc.dram_tensor("wqk", [C, 2 * C], bf16, kind="ExternalInput").ap()
    qkb = nc.dram_tensor("qkb", [128, 12], f32, kind="ExternalInput").ap()
    wv = nc.dram_tensor("wv", [C, C], bf16, kind="ExternalInput").ap()
    wp = nc.dram_tensor("wp", [C, C], bf16, kind="ExternalInput").ap()
    pb2 = nc.dram_tensor("pb2", [128, 6], f32, kind="ExternalInput").ap()
    expb = nc.dram_tensor("expb", [H, N, N], bf16, kind="ExternalInput").ap()
    out = nc.dram_tensor("out", [C, NTOK], bf16, kind="ExternalOutput").ap()

    with tile.TileContext(nc) as tc, ExitStack() as ctx:
        ent = ctx.enter_context
        const = ent(tc.tile_pool(name="const", bufs=1))
        sps = ent(tc.tile_pool(name="sps", bufs=3, space="PSUM"))
        bigp = ent(tc.tile_pool(name="bigp", bufs=1, space="PSUM"))
        xbp = ent(tc.tile_pool(name="xb", bufs=2))
        qktp = ent(tc.tile_pool(name="qkt", bufs=15))
        vpp = ent(tc.tile_pool(name="vpad", bufs=2))
        pp = ent(tc.tile_pool(name="pp", bufs=3))
        p2p = ent(tc.tile_pool(name="p2p", bufs=8))
        odsb = ent(tc.tile_pool(name="odsb", bufs=6))
        otp = ent(tc.tile_pool(name="ot", bufs=2))
        yp = ent(tc.tile_pool(name="y", bufs=2))
        smallp = ent(tc.tile_pool(name="small", bufs=2))
        small1 = ent(tc.tile_pool(name="small1", bufs=1))
        onp = ent(tc.tile_pool(name="on", bufs=2))
        rbp = ent(tc.tile_pool(name="rb", bufs=2))
        dramp = ent(tc.tile_pool(name="dram", bufs=2, space="DRAM"))

        # ---- constant tiles (DMAs emitted lazily below for overlap) ----
        wqk_sb = const.tile([128, 6, 2 * C], bf16)
        wv_sb = const.tile([128, 6, C], bf16)
        wp_sb = const.tile([128, 6, C], bf16)
        qkb_sb = const.tile([128, 12], f32)
        pb2_sb = const.tile([128, 6], f32)
        expb_sb = const.tile([128, H, 5, N], bf16)

        def load_wqk():
            for kc in range(6):
                nc.gpsimd.dma_start(wqk_sb[:, kc, :],
                                    wqk[kc * 128:(kc + 1) * 128, :])

        def load_wv_qkb():
            for kc in range(6):
                nc.gpsimd.dma_start(wv_sb[:, kc, :],
                                    wv[kc * 128:(kc + 1) * 128, :])
            nc.gpsimd.dma_start(qkb_sb[:, :], qkb[:, :])
            nc.gpsimd.dma_start(pb2_sb[:, :], pb2[:, :])

        def load_wp_expb():
            for kc in range(6):
                nc.gpsimd.dma_start(wp_sb[:, kc, :],
                                    wp[kc * 128:(kc + 1) * 128, :])
            for h in range(H):
                for jt, (j0, jsz) in enumerate(JT):
                    nc.gpsimd.dma_start(expb_sb[0:jsz, h, jt, :],
                                        expb[h, j0:j0 + jsz, :])

        # ---- per-batch state (keyed by flattened batch index) ----
        st = {}

        def emit_load_xb(i, b):
            xb = xbp.tile([128, 6, N], bf16, tag="xb")
            for kc in range(6):
                nc.sync.dma_start(xb[:, kc, :],
                                  xT[kc * 128:(kc + 1) * 128, b * N:(b + 1) * N])
            st[(i, "xb")] = xb

        def emit_qkproj(i, mt, halfk):
            xb = st[(i, "xb")]
            if halfk == 0:
                st[(i, "qps", mt)] = sps.tile([128, 768], f32, name="qps",
                                              tag="s")
            ps = st[(i, "qps", mt)]
            for kc in range(3 * halfk, 3 * halfk + 3):
                for n0, nsz in [(0, 512), (512, 64)]:
                    nc.tensor.matmul(
                        ps[:, n0:n0 + nsz],
                        lhsT=wqk_sb[:, kc, mt * 128:(mt + 1) * 128],
                        rhs=xb[:, kc, n0:n0 + nsz],
                        start=(kc == 0), stop=(kc == 5))
            if halfk == 1:
                qt = qktp.tile([128, N], bf16, tag="qkt")
                nc.vector.tensor_scalar_add(qt[:, :], ps[:, 0:N],
                                            qkb_sb[:, mt:mt + 1])
                st[(i, "qkt", mt)] = qt
                del st[(i, "qps", mt)]

        def emit_vproj(i, tt, halfk):
            xb = st[(i, "xb")]
            if tt == 0 and halfk == 0:
                vpad = vpp.tile([128, 5, H, 65], bf16, tag="vpad")
                nc.vector.memset(vpad[:, :, :, 64:65], 1.0)
                st[(i, "vpad")] = vpad
            vpad = st[(i, "vpad")]
            t0, tsz = JT[tt]
            if halfk == 0:
                st[(i, "vps", tt)] = sps.tile([128, 12, 64], f32, name="vps",
                                              tag="s")
            ps = st[(i, "vps", tt)]
            for kc in range(3 * halfk, 3 * halfk + 3):
                for g0, gn in [(0, 8), (8, 4)]:
                    nc.tensor.matmul(
                        ps[0:tsz, g0:g0 + gn, :],
                        lhsT=xb[:, kc, t0:t0 + tsz],
                        rhs=wv_sb[:, kc, g0 * 64:(g0 + gn) * 64],
                        start=(kc == 0), stop=(kc == 5))
            if halfk == 1:
                nc.vector.tensor_copy(vpad[0:tsz, tt, :, 0:64],
                                      ps[0:tsz, :, :])
                del st[(i, "vps", tt)]

        def emit_attn_qk(i, h, jt):
            base = (h % 2) * 64
            qv = st[(i, "qkt", h // 2)]
            kv = st[(i, "qkt", 6 + h // 2)]
            j0, jsz = JT[jt]
            s = sps.tile([128, 768], f32, name="satt", tag="s")
            for n0, nsz in [(0, 512), (512, 64)]:
                nc.tensor.matmul(
                    s[0:jsz, n0:n0 + nsz],
                    lhsT=kv[base:base + 64, j0:j0 + jsz],
                    rhs=qv[base:base + 64, n0:n0 + nsz],
                    start=True, stop=True)
            p1 = pp.tile([128, N], bf16, tag="p1")
            nc.scalar.activation(p1[0:jsz, :], s[0:jsz, 0:N], EXP)
            p2 = p2p.tile([128, N], bf16, tag="p2")
            nc.vector.tensor_mul(p2[0:jsz, :], p1[0:jsz, :],
                                 expb_sb[0:jsz, h, jt, :])
            st[(i, "p2", h, jt)] = p2

        def emit_attn_pv(i, h, jt):
            vpad = st[(i, "vpad")]
            j0, jsz = JT[jt]
            if jt == 0:
                st[(i, "od")] = bigp.tile([65, N], f32, name="od", tag="od")
            odp = st[(i, "od")]
            p2 = st.pop((i, "p2", h, jt))
            for n0, nsz in [(0, 512), (512, 64)]:
                nc.tensor.matmul(
                    odp[0:65, n0:n0 + nsz],
                    lhsT=vpad[0:jsz, jt, h, :],
                    rhs=p2[0:jsz, n0:n0 + nsz],
                    start=(jt == 0), stop=(jt == 4),
                    skip_group_check=True)
            if jt == 4:
                hh = h % 6
                if hh == 0:
                    st[(i, "d6", h // 6)] = smallp.tile([6, N], bf16, name="d6", tag="d6")
                d6 = st[(i, "d6", h // 6)]
                ods = odsb.tile([65, N], bf16, tag="ods")
                nc.vector.tensor_copy(ods[:, :], odp[:, :])
                nc.sync.dma_start(d6[hh:hh + 1, :], ods[64:65, :])
                st[(i, "ods", h)] = ods

        def emit_division(i, half):
            d6 = st[(i, "d6", half)]
            if half == 0:
                ot = otp.tile([128, 6, N], bf16, tag="ot")
                st[(i, "ot")] = ot
            ot = st[(i, "ot")]
            df32 = small1.tile([6, N], f32, tag="df32")
            nc.vector.tensor_copy(df32[:, :], d6[:, :])
            rf32 = small1.tile([6, N], f32, tag="rf32")
            nc.vector.reciprocal(rf32[:, :], df32[:, :])
            rbf = small1.tile([6, N], bf16, tag="rbf")
            nc.vector.tensor_copy(rbf[:, :], rf32[:, :])
            rdram = dramp.tile([6, N], bf16, tag="rdram")
            nc.sync.dma_start(rdram[:, :], rbf[:, :])
            for hh in range(6):
                h = half * 6 + hh
                rb = rbp.tile([64, N], bf16, tag="rb")
                nc.sync.dma_start(rb[:, :],
                                  rdram[hh:hh + 1, :].partition_broadcast(64))
                on = onp.tile([64, N], bf16, tag="on")
                nc.vector.tensor_mul(on[:, :], st[(i, "ods", h)][0:64, :],
                                     rb[:, :])
                nc.sync.dma_start(
                    ot[(h % 2) * 64:(h % 2) * 64 + 64, h // 2, :], on[:, :])

        def emit_proj(i, b, mt, halfk):
            ot = st[(i, "ot")]
            if halfk == 0:
                st[(i, "pps", mt)] = sps.tile([128, 768], f32, name="pps",
                                              tag="s")
            ps = st[(i, "pps", mt)]
            for kc in range(3 * halfk, 3 * halfk + 3):
                for n0, nsz in [(0, 512), (512, 64)]:
                    nc.tensor.matmul(
                        ps[:, n0:n0 + nsz],
                        lhsT=wp_sb[:, kc, mt * 128:(mt + 1) * 128],
                        rhs=ot[:, kc, n0:n0 + nsz],
                        start=(kc == 0), stop=(kc == 5))
            if halfk == 1:
                y = yp.tile([128, N], bf16, tag="y")
                nc.vector.tensor_scalar_add(y[:, :], ps[:, 0:N],
                                            pb2_sb[:, mt:mt + 1])
                nc.sync.dma_start(
                    out[mt * 128:(mt + 1) * 128, b * N:(b + 1) * N], y[:, :])
                del st[(i, "pps", mt)]

        # ---- pipelined emission ----
        from collections import deque
        batches = [(r * BL + b, b) for r in range(reps) for b in range(BL)]
        nbat = len(batches)

        load_wqk()
        load_wv_qkb()
        emit_load_xb(0, 0)
        for mt in range(12):
            emit_qkproj(0, mt, 0)
            emit_qkproj(0, mt, 1)
        for tt in range(5):
            emit_vproj(0, tt, 0)
            emit_vproj(0, tt, 1)
        load_wp_expb()

        for idx, (i, b) in enumerate(batches):
            pre = deque()
            qkq = deque()
            if idx + 1 < nbat:
                i2, b2 = batches[idx + 1]
                pre.append(lambda i2=i2, b2=b2: emit_load_xb(i2, b2))
            if idx > 0:
                ip, bp = batches[idx - 1]
                for mt in range(6):
                    pre.append(lambda ip=ip, bp=bp, mt=mt:
                               (emit_proj(ip, bp, mt, 0),
                                emit_proj(ip, bp, mt, 1)))
            if idx + 1 < nbat:
                i2, b2 = batches[idx + 1]
                for tt in range(5):
                    pre.append(lambda i2=i2, tt=tt:
                               (emit_vproj(i2, tt, 0), emit_vproj(i2, tt, 1)))
                for mt in range(12):
                    qkq.append(lambda i2=i2, mt=mt:
                               (emit_qkproj(i2, mt, 0), emit_qkproj(i2, mt, 1)))
            for half in range(2):
                for hp in range(3):
                    he = half * 6 + 2 * hp
                    ho = he + 1
                    for jt in range(5):
                        emit_attn_qk(i, he, jt)
                        emit_attn_qk(i, ho, jt)
                        emit_attn_pv(i, he, jt)
                    if pre:
                        pre.popleft()()
                    for jt in range(5):
                        emit_attn_pv(i, ho, jt)
                    if qkq:
                        qkq.popleft()()
                        qkq.popleft()()
                emit_division(i, half)
            while pre:
                pre.popleft()()
            while qkq:
                qkq.popleft()()
        ilast, blast = batches[-1]
        for mt in range(6):
            emit_proj(ilast, blast, mt, 0)
            emit_proj(ilast, blast, mt, 1)

    nc.compile()
    _cache[key] = nc
    return nc


def _prep_inputs(x, qkv_w, q_bias, v_bias, rpb_table, rel_idx, proj_w, proj_b):
    x = np.asarray(x, np.float32)
    qkv_w = np.asarray(qkv_w, np.float32)
    q_bias = np.asarray(q_bias, np.float32)
    v_bias = np.asarray(v_bias, np.float32)
    rpb_table = np.asarray(rpb_table, np.float32)
    rel_idx = np.asarray(rel_idx)
    proj_w = np.asarray(proj_w, np.float32)
    proj_b = np.asarray(proj_b, np.float32)

    scale = D ** -0.5
    wqk_f = qkv_w[:2 * C].T.copy()            # [768, 1536]
    wqk_f[:, :C] *= scale
    wqk_np = wqk_f.astype(BF16)
    qb = np.concatenate([q_bias * scale, np.zeros(C, np.float32)])  # [1536]
    qkb_np = np.ascontiguousarray(qb.reshape(12, 128).T)            # [128, 12]
    wv_np = qkv_w[2 * C:].T.copy().astype(BF16)
    wp_np = proj_w.T.copy().astype(BF16)
    pb_eff = (proj_b + proj_w @ v_bias).astype(np.float32)          # [768]
    pb2_np = np.ascontiguousarray(pb_eff.reshape(6, 128).T)         # [128, 6]
    et = np.exp(rpb_table)                     # [2209, 12]
    idx = np.clip(np.asarray(rel_idx, np.int64), 0, et.shape[0] - 1)
    g = et[idx]                                # [576i, 576j, 12]
    expb_np = np.ascontiguousarray(g.transpose(2, 1, 0)).astype(BF16)

    in_maps = []
    for ci in range(NCORES):
        xc = x[ci * BL:(ci + 1) * BL]          # [8, 576, 768]
        xT_np = np.ascontiguousarray(
            xc.transpose(2, 0, 1).reshape(C, NTOK)).astype(BF16)
        in_maps.append({
            "xT": xT_np, "wqk": wqk_np, "qkb": qkb_np,
            "wv": wv_np, "wp": wp_np, "pb2": pb2_np, "expb": expb_np,
        })
    return in_maps


def kernel(x, qkv_w, q_bias, v_bias, rpb_table, rel_idx, proj_w, proj_b,
           _want_profile=False):
    in_maps = _prep_inputs(x, qkv_w, q_bias, v_bias, rpb_table, rel_idx,
                           proj_w, proj_b)
    nc = _build()
    from concourse.bass_utils import run_bass_kernel_spmd
    res = run_bass_kernel_spmd(nc, in_maps, core_ids=list(range(NCORES)),
                               trace=_want_profile)
    outs = [np.asarray(r["out"], np.float32).T.reshape(BL, N, C)
            for r in res.results]
    y = np.concatenate(outs, 0)
    if _want_profile:
        return y, res
    return y


# revision 44
# speedup vs baseline: 1.0724x; 1.0724x over previous
"""Trainium2 Bass kernel: Swin-style window attention with relative position bias.

Self-contained: hardcodes B=64, N=576, C=768, H=12. Shards batch over 8 cores.

Per-core design (fully on-device compute; host only reshapes/sharding):
 - qk projection emitted in transposed layout qkT[feat, tok] (attention scale
   folded into q weights), v in normal layout [tok, feat] padded with a ones
   column per head (softmax denominator trick).
 - S_T[j,i] = k^T q computed per (batch, head); softmax without max-subtract
   (logits are bounded): P = exp(S_T) * exp(bias_T), bias exp-table built on
   host from rpb_table[rel_idx] (works for arbitrary rel_idx).
 - PV: oD[65, i] = [v | 1]^T P_T accumulated over j-chunks -> row 64 holds the
   softmax denominator. Reciprocal batched per batch, broadcast via stride-0
   DMA, multiply, assemble oT, project with bias via ones-row.
"""
import sys

sys.path.insert(0, "/opt/trn_rl_repo")

import numpy as np
import ml_dtypes

BF16 = ml_dtypes.bfloat16

B, N, C = 64, 576, 768
H, D = 12, 64
NCORES = 8
BL = B // NCORES           # 8 batches per core
NTOK = BL * N              # 4608 tokens per core

# token/j tiles of N=576: 4x128 + 1x64
JT = [(0, 128), (128, 128), (256, 128), (384, 128), (512, 64)]

_cache = {}


def _build(reps=1):
    key = ("nc", reps)
    if key in _cache:
        return _cache[key]
    from contextlib import ExitStack
    import concourse.tile as tile
    from concourse import bacc, mybir

    f32 = mybir.dt.float32
    bf16 = mybir.dt.bfloat16
    EXP = mybir.ActivationFunctionType.Exp

    nc = bacc.Bacc("TRN2", target_bir_lowering=False, debug=False,
                   num_devices=NCORES)
    xT = nc.dram_tensor("xT", [C, NTOK], bf16, kind="ExternalInput").ap()
    wqk = nc.dram_tensor("wqk", [C, 2 * C], bf16, kind="ExternalInput").ap()
    qkb = nc.dram_tensor("qkb", [128, 12], f32, kind="ExternalInput").ap()
    wv = nc.dram_tensor("wv", [C, C], bf16, kind="ExternalInput").ap()
    wp = nc.dram_tensor("wp", [C, C], bf16, kind="ExternalInput").ap()
    pb2 = nc.dram_tensor("pb2", [128, 6], f32, kind="ExternalInput").ap()
    expb = nc.dram_tensor("expb", [H, N, N], bf16, kind="ExternalInput").ap()
    out = nc.dram_tensor("out", [C, NTOK], bf16, kind="ExternalOutput").ap()

    with tile.TileContext(nc) as tc, ExitStack() as ctx:
        ent = ctx.enter_context
        const = ent(tc.tile_pool(name="const", bufs=1))
        sps = ent(tc.tile_pool(name="sps", bufs=3, space="PSUM"))
        bigp = ent(tc.tile_pool(name="bigp", bufs=1, space="PSUM"))
        xbp = ent(tc.tile_pool(name="xb", bufs=2))
        qktp = ent(tc.tile_pool(name="qkt", bufs=15))
        vpp = ent(tc.tile_pool(name="vpad", bufs=2))
        pp = ent(tc.tile_pool(name="pp", bufs=3))
        p2p = ent(tc.tile_pool(name="p2p", bufs=8))
        odsb = ent(tc.tile_pool(name="odsb", bufs=6))
        otp = ent(tc.tile_pool(name="ot", bufs=2))
        yp = ent(tc.tile_pool(name="y", bufs=2))
        smallp = ent(tc.tile_pool(name="small", bufs=2))
        small1 = ent(tc.tile_pool(name="small1", bufs=1))
        onp = ent(tc.tile_pool(name="on", bufs=2))
        rbp = ent(tc.tile_pool(name="rb", bufs=2))
        dramp = ent(tc.tile_pool(name="dram", bufs=2, space="DRAM"))

        # ---- constant tiles (DMAs emitted lazily below for overlap) ----
        wqk_sb = const.tile([128, 6, 2 * C], bf16)
        wv_sb = const.tile([128, 6, C], bf16)
        wp_sb = const.tile([128, 6, C], bf16)
        qkb_sb = const.tile([128, 12], f32)
        pb2_sb = const.tile([128, 6], f32)
        expb_sb = const.tile([128, H, 5, N], bf16)

        def load_wqk():
            for kc in range(6):
                nc.gpsimd.dma_start(wqk_sb[:, kc, :],
                                    wqk[kc * 128:(kc + 1) * 128, :])

        def load_wv_qkb():
            for kc in range(6):
                nc.gpsimd.dma_start(wv_sb[:, kc, :],
                                    wv[kc * 128:(kc + 1) * 128, :])
            nc.gpsimd.dma_start(qkb_sb[:, :], qkb[:, :])
            nc.gpsimd.dma_start(pb2_sb[:, :], pb2[:, :])

        def load_wp_expb():
            for kc in range(6):
                nc.gpsimd.dma_start(wp_sb[:, kc, :],
                                    wp[kc * 128:(kc + 1) * 128, :])
            for h in range(H):
                for jt, (j0, jsz) in enumerate(JT):
                    nc.gpsimd.dma_start(expb_sb[0:jsz, h, jt, :],
                                        expb[h, j0:j0 + jsz, :])

        # ---- per-batch state (keyed by flattened batch index) ----
        st = {}

        def emit_load_xb(i, b):
            xb = xbp.tile([128, 6, N], bf16, tag="xb")
            for kc in range(6):
                nc.sync.dma_start(xb[:, kc, :],
                                  xT[kc * 128:(kc + 1) * 128, b * N:(b + 1) * N])
            st[(i, "xb")] = xb

        def emit_qkproj(i, mt, halfk):
            xb = st[(i, "xb")]
            if halfk == 0:
                st[(i, "qps", mt)] = sps.tile([128, 768], f32, name="qps",
                                              tag="s")
            ps = st[(i, "qps", mt)]
            for kc in range(3 * halfk, 3 * halfk + 3):
                for n0, nsz in [(0, 512), (512, 64)]:
                    nc.tensor.matmul(
                        ps[:, n0:n0 + nsz],
                        lhsT=wqk_sb[:, kc, mt * 128:(mt + 1) * 128],
                        rhs=xb[:, kc, n0:n0 + nsz],
                        start=(kc == 0), stop=(kc == 5))
            if halfk == 1:
                qt = qktp.tile([128, N], bf16, tag="qkt")
                nc.vector.tensor_scalar_add(qt[:, :], ps[:, 0:N],
                                            qkb_sb[:, mt:mt + 1])
                st[(i, "qkt", mt)] = qt
                del st[(i, "qps", mt)]

        def emit_vproj(i, tt, halfk):
            xb = st[(i, "xb")]
            if tt == 0 and halfk == 0:
                vpad = vpp.tile([128, 5, H, 65], bf16, tag="vpad")
                nc.vector.memset(vpad[:, :, :, 64:65], 1.0)
                st[(i, "vpad")] = vpad
            vpad = st[(i, "vpad")]
            t0, tsz = JT[tt]
            if halfk == 0:
                st[(i, "vps", tt)] = sps.tile([128, 12, 64], f32, name="vps",
                                              tag="s")
            ps = st[(i, "vps", tt)]
            for kc in range(3 * halfk, 3 * halfk + 3):
                for g0, gn in [(0, 8), (8, 4)]:
                    nc.tensor.matmul(
                        ps[0:tsz, g0:g0 + gn, :],
                        lhsT=xb[:, kc, t0:t0 + tsz],
                        rhs=wv_sb[:, kc, g0 * 64:(g0 + gn) * 64],
                        start=(kc == 0), stop=(kc == 5))
            if halfk == 1:
                nc.vector.tensor_copy(vpad[0:tsz, tt, :, 0:64],
                                      ps[0:tsz, :, :])
                del st[(i, "vps", tt)]

        def emit_attn_qk(i, h, jt):
            base = (h % 2) * 64
            qv = st[(i, "qkt", h // 2)]
            kv = st[(i, "qkt", 6 + h // 2)]
            j0, jsz = JT[jt]
            s = sps.tile([128, 768], f32, name="satt", tag="s")
            for n0, nsz in [(0, 512), (512, 64)]:
                nc.tensor.matmul(
                    s[0:jsz, n0:n0 + nsz],
                    lhsT=kv[base:base + 64, j0:j0 + jsz],
                    rhs=qv[base:base + 64, n0:n0 + nsz],
                    start=True, stop=True)
            p1 = pp.tile([128, N], bf16, tag="p1")
            nc.scalar.activation(p1[0:jsz, :], s[0:jsz, 0:N], EXP)
            p2 = p2p.tile([128, N], bf16, tag="p2")
            nc.vector.tensor_mul(p2[0:jsz, :], p1[0:jsz, :],
                                 expb_sb[0:jsz, h, jt, :])
            st[(i, "p2", h, jt)] = p2

        def emit_attn_pv(i, h, jt):
            vpad = st[(i, "vpad")]
            j0, jsz = JT[jt]
            if jt == 0:
                st[(i, "od")] = bigp.tile([65, N], f32, name="od", tag="od")
            odp = st[(i, "od")]
            p2 = st.pop((i, "p2", h, jt))
            for n0, nsz in [(0, 512), (512, 64)]:
                nc.tensor.matmul(
                    odp[0:65, n0:n0 + nsz],
                    lhsT=vpad[0:jsz, jt, h, :],
                    rhs=p2[0:jsz, n0:n0 + nsz],
                    start=(jt == 0), stop=(jt == 4),
                    skip_group_check=True)
            if jt == 4:
                hh = h % 6
                if hh == 0:
                    st[(i, "d6", h // 6)] = smallp.tile([6, N], bf16, name="d6", tag="d6")
                d6 = st[(i, "d6", h // 6)]
                ods = odsb.tile([65, N], bf16, tag="ods")
                nc.vector.tensor_copy(ods[:, :], odp[:, :])
                nc.sync.dma_start(d6[hh:hh + 1, :], ods[64:65, :])
                st[(i, "ods", h)] = ods

        def emit_division(i, half):
            d6 = st[(i, "d6", half)]
            if half == 0:
                ot = otp.tile([128, 6, N], bf16, tag="ot")
                st[(i, "ot")] = ot
            ot = st[(i, "ot")]
            df32 = small1.tile([6, N], f32, tag="df32")
            nc.vector.tensor_copy(df32[:, :], d6[:, :])
            rf32 = small1.tile([6, N], f32, tag="rf32")
            nc.vector.reciprocal(rf32[:, :], df32[:, :])
            rbf = small1.tile([6, N], bf16, tag="rbf")
            nc.vector.tensor_copy(rbf[:, :], rf32[:, :])
            rdram = dramp.tile([6, N], bf16, tag="rdram")
            nc.sync.dma_start(rdram[:, :], rbf[:, :])
            for hh in range(6):
                h = half * 6 + hh
                rb = rbp.tile([64, N], bf16, tag="rb")
                nc.sync.dma_start(rb[:, :],
                                  rdram[hh:hh + 1, :].partition_broadcast(64))
                on = onp.tile([64, N], bf16, tag="on")
                nc.vector.tensor_mul(on[:, :], st[(i, "ods", h)][0:64, :],
                                     rb[:, :])
                nc.sync.dma_start(
                    ot[(h % 2) * 64:(h % 2) * 64 + 64, h // 2, :], on[:, :])

        def emit_proj(i, b, mt, halfk):
            ot = st[(i, "ot")]
            if halfk == 0:
                st[(i, "pps", mt)] = sps.tile([128, 768], f32, name="pps",
                                              tag="s")
            ps = st[(i, "pps", mt)]
            for kc in range(3 * halfk, 3 * halfk + 3):
                for n0, nsz in [(0, 512), (512, 64)]:
                    nc.tensor.matmul(
                        ps[:, n0:n0 + nsz],
                        lhsT=wp_sb[:, kc, mt * 128:(mt + 1) * 128],
                        rhs=ot[:, kc, n0:n0 + nsz],
                        start=(kc == 0), stop=(kc == 5))
            if halfk == 1:
                y = yp.tile([128, N], bf16, tag="y")
                nc.vector.tensor_scalar_add(y[:, :], ps[:, 0:N],
                                            pb2_sb[:, mt:mt + 1])
                nc.sync.dma_start(
                    out[mt * 128:(mt + 1) * 128, b * N:(b + 1) * N], y[:, :])
                del st[(i, "pps", mt)]

        # ---- pipelined emission ----
        from collections import deque
        batches = [(r * BL + b, b) for r in range(reps) for b in range(BL)]
        nbat = len(batches)

        load_wqk()
        load_wv_qkb()
        emit_load_xb(0, 0)
        for mt in range(12):
            emit_qkproj(0, mt, 0)
            emit_qkproj(0, mt, 1)
        for tt in range(5):
            emit_vproj(0, tt, 0)
            emit_vproj(0, tt, 1)
        load_wp_expb()

        for idx, (i, b) in enumerate(batches):
            pre = deque()
            qkq = deque()
            if idx + 1 < nbat:
                i2, b2 = batches[idx + 1]
                pre.append(lambda i2=i2, b2=b2: emit_load_xb(i2, b2))
            if idx > 0:
                ip, bp = batches[idx - 1]
                for mt in range(6):
                    pre.append(lambda ip=ip, bp=bp, mt=mt:
                               (emit_proj(ip, bp, mt, 0),
                                emit_proj(ip, bp, mt, 1)))
            if idx + 1 < nbat:
                i2, b2 = batches[idx + 1]
                for tt in range(5):
                    pre.append(lambda i2=i2, tt=tt:
                               (emit_vproj(i2, tt, 0), emit_vproj(i2, tt, 1)))
                for mt in range(12):
                    qkq.append(lambda i2=i2, mt=mt:
                               (emit_qkproj(i2, mt, 0), emit_qkproj(i2, mt, 1)))
            for half in range(2):
                for hp in range(3):
                    he = half * 6 + 2 * hp
                    ho = he + 1
                    for jt in range(5):
                        emit_attn_qk(i, he, jt)
                        emit_attn_qk(i, ho, jt)
                        emit_attn_pv(i, he, jt)
                    if pre:
                        pre.popleft()()
                    for jt in range(5):
                        emit_attn_pv(i, ho, jt)
                    if qkq:
                        qkq.popleft()()
                        qkq.popleft()()
                emit_division(i, half)
            while pre:
                pre.popleft()()
            while qkq:
                qkq.popleft()()
        ilast, blast = batches[-1]
        for mt in range(6):
            emit_proj(ilast, blast, mt, 0)
            emit_proj(ilast, blast, mt, 1)

    nc.compile()
    _cache[key] = nc
    return nc


def _prep_inputs(x, qkv_w, q_bias, v_bias, rpb_table, rel_idx, proj_w, proj_b):
    x = np.asarray(x, np.float32)
    qkv_w = np.asarray(qkv_w, np.float32)
    q_bias = np.asarray(q_bias, np.float32)
    v_bias = np.asarray(v_bias, np.float32)
    rpb_table = np.asarray(rpb_table, np.float32)
    rel_idx = np.asarray(rel_idx)
    proj_w = np.asarray(proj_w, np.float32)
    proj_b = np.asarray(proj_b, np.float32)

    scale = D ** -0.5
    wqk_f = qkv_w[:2 * C].T.copy()            # [768, 1536]
    wqk_f[:, :C] *= scale
    wqk_np = wqk_f.astype(BF16)
    qb = np.concatenate([q_bias * scale, np.zeros(C, np.float32)])  # [1536]
    qkb_np = np.ascontiguousarray(qb.reshape(12, 128).T)            # [128, 12]
    wv_np = qkv_w[2 * C:].T.copy().astype(BF16)
    wp_np = proj_w.T.copy().astype(BF16)
    pb_eff = (proj_b + proj_w @ v_bias).astype(np.float32)          # [768]
    pb2_np = np.ascontiguousarray(pb_eff.reshape(6, 128).T)         # [128, 6]
    et = np.exp(rpb_table)                     # [2209, 12]
    idx = np.clip(np.asarray(rel_idx, np.int64), 0, et.shape[0] - 1)
    g = et[idx]                                # [576i, 576j, 12]
    expb_np = np.ascontiguousarray(g.transpose(2, 1, 0)).astype(BF16)

    in_maps = []
    for ci in range(NCORES):
        xc = x[ci * BL:(ci + 1) * BL]          # [8, 576, 768]
        xT_np = np.ascontiguousarray(
            xc.transpose(2, 0, 1).reshape(C, NTOK)).astype(BF16)
        in_maps.append({
            "xT": xT_np, "wqk": wqk_np, "qkb": qkb_np,
            "wv": wv_np, "wp": wp_np, "pb2": pb2_np, "expb": expb_np,
        })
    return in_maps


def kernel(x, qkv_w, q_bias, v_bias, rpb_table, rel_idx, proj_w, proj_b,
           _want_profile=False):
    in_maps = _prep_inputs(x, qkv_w, q_bias, v_bias, rpb_table, rel_idx,
                           proj_w, proj_b)
    nc = _build()
    from concourse.bass_utils import run_bass_kernel_spmd
    res = run_bass_kernel_spmd(nc, in_maps, core_ids=list(range(NCORES)),
                               trace=_want_profile)
    outs = [np.asarray(r["out"], np.float32).T.reshape(BL, N, C)
            for r in res.results]
    y = np.concatenate(outs, 0)
    if _want_profile:
        return y, res
    return y


# revision 46
# speedup vs baseline: 1.1621x; 1.0836x over previous
"""Trainium2 Bass kernel: Swin-style window attention with relative position bias.

Self-contained: hardcodes B=64, N=576, C=768, H=12. Shards batch over 8 cores.

Per-core design (fully on-device compute; host only reshapes/sharding):
 - qk projection emitted in transposed layout qkT[feat, tok] (attention scale
   folded into q weights), v in normal layout [tok, feat] padded with a ones
   column per head (softmax denominator trick).
 - S_T[j,i] = k^T q computed per (batch, head); softmax without max-subtract
   (logits are bounded): P = exp(S_T) * exp(bias_T), bias exp-table built on
   host from rpb_table[rel_idx] (works for arbitrary rel_idx).
 - PV: oD[65, i] = [v | 1]^T P_T accumulated over j-chunks -> row 64 holds the
   softmax denominator. Reciprocal batched per batch, broadcast via stride-0
   DMA, multiply, assemble oT, project with bias via ones-row.
"""
import sys

sys.path.insert(0, "/opt/trn_rl_repo")

import numpy as np
import ml_dtypes

BF16 = ml_dtypes.bfloat16

B, N, C = 64, 576, 768
H, D = 12, 64
NCORES = 8
BL = B // NCORES           # 8 batches per core
NTOK = BL * N              # 4608 tokens per core

# token/j tiles of N=576: 4x128 + 1x64
JT = [(0, 128), (128, 128), (256, 128), (384, 128), (512, 64)]

_cache = {}


def _build(reps=1):
    key = ("nc", reps)
    if key in _cache:
        return _cache[key]
    from contextlib import ExitStack
    import concourse.tile as tile
    from concourse import bacc, mybir

    f32 = mybir.dt.float32
    bf16 = mybir.dt.bfloat16
    EXP = mybir.ActivationFunctionType.Exp

    nc = bacc.Bacc("TRN2", target_bir_lowering=False, debug=False,
                   num_devices=NCORES)
    xT = nc.dram_tensor("xT", [C, NTOK], bf16, kind="ExternalInput").ap()
    wqk = nc.dram_tensor("wqk", [C, 2 * C], bf16, kind="ExternalInput").ap()
    qkb = nc.dram_tensor("qkb", [128, 12], f32, kind="ExternalInput").ap()
    wv = nc.dram_tensor("wv", [C, C], bf16, kind="ExternalInput").ap()
    wp = nc.dram_tensor("wp", [C, C], bf16, kind="ExternalInput").ap()
    pb2 = nc.dram_tensor("pb2", [128, 6], f32, kind="ExternalInput").ap()
    expb = nc.dram_tensor("expb", [H, N, N], bf16, kind="ExternalInput").ap()
    out = nc.dram_tensor("out", [C, NTOK], bf16, kind="ExternalOutput").ap()

    with tile.TileContext(nc) as tc, ExitStack() as ctx:
        ent = ctx.enter_context
        const = ent(tc.tile_pool(name="const", bufs=1))
        sps = ent(tc.tile_pool(name="sps", bufs=3, space="PSUM"))
        bigp = ent(tc.tile_pool(name="bigp", bufs=1, space="PSUM"))
        xbp = ent(tc.tile_pool(name="xb", bufs=2))
        qktp = ent(tc.tile_pool(name="qkt", bufs=15))
        vpp = ent(tc.tile_pool(name="vpad", bufs=2))
        pp = ent(tc.tile_pool(name="pp", bufs=3))
        p2p = ent(tc.tile_pool(name="p2p", bufs=8))
        odsb = ent(tc.tile_pool(name="odsb", bufs=6))
        otp = ent(tc.tile_pool(name="ot", bufs=2))
        yp = ent(tc.tile_pool(name="y", bufs=2))
        smallp = ent(tc.tile_pool(name="small", bufs=2))
        small1 = ent(tc.tile_pool(name="small1", bufs=1))
        onp = ent(tc.tile_pool(name="on", bufs=2))
        rbp = ent(tc.tile_pool(name="rb", bufs=2))
        dramp = ent(tc.tile_pool(name="dram", bufs=2, space="DRAM"))

        # ---- constant tiles (DMAs emitted lazily below for overlap) ----
        wqk_sb = const.tile([128, 6, 2 * C], bf16)
        wv_sb = const.tile([128, 6, C], bf16)
        wp_sb = const.tile([128, 6, C], bf16)
        qkb_sb = const.tile([128, 12], f32)
        pb2_sb = const.tile([128, 6], f32)
        expb_sb = const.tile([128, H, 5, N], bf16)

        def load_wqk():
            for kc in range(6):
                nc.gpsimd.dma_start(wqk_sb[:, kc, :],
                                    wqk[kc * 128:(kc + 1) * 128, :])

        def load_wv_qkb():
            for kc in range(6):
                nc.gpsimd.dma_start(wv_sb[:, kc, :],
                                    wv[kc * 128:(kc + 1) * 128, :])
            nc.gpsimd.dma_start(qkb_sb[:, :], qkb[:, :])
            nc.gpsimd.dma_start(pb2_sb[:, :], pb2[:, :])

        def load_wp_expb():
            for kc in range(6):
                nc.gpsimd.dma_start(wp_sb[:, kc, :],
                                    wp[kc * 128:(kc + 1) * 128, :])
            for h in range(H):
                for jt, (j0, jsz) in enumerate(JT):
                    nc.gpsimd.dma_start(expb_sb[0:jsz, h, jt, :],
                                        expb[h, j0:j0 + jsz, :])

        # ---- per-batch state (keyed by flattened batch index) ----
        st = {}

        def emit_load_xb(i, b):
            xb = xbp.tile([128, 6, N], bf16, tag="xb")
            for kc in range(6):
                nc.sync.dma_start(xb[:, kc, :],
                                  xT[kc * 128:(kc + 1) * 128, b * N:(b + 1) * N])
            st[(i, "xb")] = xb

        def emit_qkproj(i, mt, halfk):
            xb = st[(i, "xb")]
            if halfk == 0:
                st[(i, "qps", mt)] = sps.tile([128, 768], f32, name="qps",
                                              tag="s")
            ps = st[(i, "qps", mt)]
            for kc in range(3 * halfk, 3 * halfk + 3):
                for n0, nsz in [(0, 512), (512, 64)]:
                    nc.tensor.matmul(
                        ps[:, n0:n0 + nsz],
                        lhsT=wqk_sb[:, kc, mt * 128:(mt + 1) * 128],
                        rhs=xb[:, kc, n0:n0 + nsz],
                        start=(kc == 0), stop=(kc == 5))
            if halfk == 1:
                qt = qktp.tile([128, N], bf16, tag="qkt")
                nc.vector.tensor_scalar_add(qt[:, :], ps[:, 0:N],
                                            qkb_sb[:, mt:mt + 1])
                st[(i, "qkt", mt)] = qt
                del st[(i, "qps", mt)]

        def emit_vproj(i, tt, halfk):
            xb = st[(i, "xb")]
            if tt == 0 and halfk == 0:
                vpad = vpp.tile([128, 5, H, 65], bf16, tag="vpad")
                nc.vector.memset(vpad[:, :, :, 64:65], 1.0)
                st[(i, "vpad")] = vpad
            vpad = st[(i, "vpad")]
            t0, tsz = JT[tt]
            if halfk == 0:
                st[(i, "vps", tt)] = sps.tile([128, 12, 64], f32, name="vps",
                                              tag="s")
            ps = st[(i, "vps", tt)]
            for kc in range(3 * halfk, 3 * halfk + 3):
                for g0, gn in [(0, 8), (8, 4)]:
                    nc.tensor.matmul(
                        ps[0:tsz, g0:g0 + gn, :],
                        lhsT=xb[:, kc, t0:t0 + tsz],
                        rhs=wv_sb[:, kc, g0 * 64:(g0 + gn) * 64],
                        start=(kc == 0), stop=(kc == 5))
            if halfk == 1:
                nc.vector.tensor_copy(vpad[0:tsz, tt, :, 0:64],
                                      ps[0:tsz, :, :])
                del st[(i, "vps", tt)]

        def emit_attn_qk(i, h, jt):
            base = (h % 2) * 64
            qv = st[(i, "qkt", h // 2)]
            kv = st[(i, "qkt", 6 + h // 2)]
            j0, jsz = JT[jt]
            s = sps.tile([128, 768], f32, name="satt", tag="s")
            for n0, nsz in [(0, 512), (512, 64)]:
                nc.tensor.matmul(
                    s[0:jsz, n0:n0 + nsz],
                    lhsT=kv[base:base + 64, j0:j0 + jsz],
                    rhs=qv[base:base + 64, n0:n0 + nsz],
                    start=True, stop=True)
            p1 = pp.tile([128, N], bf16, tag="p1")
            nc.scalar.activation(p1[0:jsz, :], s[0:jsz, 0:N], EXP)
            p2 = p2p.tile([128, N], bf16, tag="p2")
            nc.vector.tensor_mul(p2[0:jsz, :], p1[0:jsz, :],
                                 expb_sb[0:jsz, h, jt, :])
            st[(i, "p2", h, jt)] = p2

        def emit_attn_pv(i, h, jt):
            vpad = st[(i, "vpad")]
            j0, jsz = JT[jt]
            if jt == 0:
                st[(i, "od")] = bigp.tile([65, N], f32, name="od", tag="od")
            odp = st[(i, "od")]
            p2 = st.pop((i, "p2", h, jt))
            for n0, nsz in [(0, 512), (512, 64)]:
                nc.tensor.matmul(
                    odp[0:65, n0:n0 + nsz],
                    lhsT=vpad[0:jsz, jt, h, :],
                    rhs=p2[0:jsz, n0:n0 + nsz],
                    start=(jt == 0), stop=(jt == 4),
                    skip_group_check=True)
            if jt == 4:
                hh = h % 6
                if hh == 0:
                    st[(i, "d6", h // 6)] = smallp.tile([6, N], bf16, name="d6", tag="d6")
                d6 = st[(i, "d6", h // 6)]
                ods = odsb.tile([65, N], bf16, tag="ods")
                nc.vector.tensor_copy(ods[:, :], odp[:, :])
                nc.sync.dma_start(d6[hh:hh + 1, :], ods[64:65, :])
                st[(i, "ods", h)] = ods

        def emit_division(i, half):
            d6 = st[(i, "d6", half)]
            if half == 0:
                ot = otp.tile([128, 6, N], bf16, tag="ot")
                st[(i, "ot")] = ot
            ot = st[(i, "ot")]
            df32 = small1.tile([6, N], f32, tag="df32")
            nc.vector.tensor_copy(df32[:, :], d6[:, :])
            rf32 = small1.tile([6, N], f32, tag="rf32")
            nc.vector.reciprocal(rf32[:, :], df32[:, :])
            rbf = small1.tile([6, N], bf16, tag="rbf")
            nc.vector.tensor_copy(rbf[:, :], rf32[:, :])
            rdram = dramp.tile([6, N], bf16, tag="rdram")
            nc.sync.dma_start(rdram[:, :], rbf[:, :])
            for hh in range(6):
                h = half * 6 + hh
                rb = rbp.tile([64, N], bf16, tag="rb")
                nc.sync.dma_start(rb[:, :],
                                  rdram[hh:hh + 1, :].partition_broadcast(64))
                on = onp.tile([64, N], bf16, tag="on")
                nc.vector.tensor_mul(on[:, :], st[(i, "ods", h)][0:64, :],
                                     rb[:, :])
                nc.sync.dma_start(
                    ot[(h % 2) * 64:(h % 2) * 64 + 64, h // 2, :], on[:, :])

        def emit_proj(i, b, mt, halfk):
            ot = st[(i, "ot")]
            if halfk == 0:
                st[(i, "pps", mt)] = sps.tile([128, 768], f32, name="pps",
                                              tag="s")
            ps = st[(i, "pps", mt)]
            for kc in range(3 * halfk, 3 * halfk + 3):
                for n0, nsz in [(0, 512), (512, 64)]:
                    nc.tensor.matmul(
                        ps[:, n0:n0 + nsz],
                        lhsT=wp_sb[:, kc, mt * 128:(mt + 1) * 128],
                        rhs=ot[:, kc, n0:n0 + nsz],
                        start=(kc == 0), stop=(kc == 5))
            if halfk == 1:
                y = yp.tile([128, N], bf16, tag="y")
                nc.vector.tensor_scalar_add(y[:, :], ps[:, 0:N],
                                            pb2_sb[:, mt:mt + 1])
                nc.sync.dma_start(
                    out[mt * 128:(mt + 1) * 128, b * N:(b + 1) * N], y[:, :])
                del st[(i, "pps", mt)]

        # ---- pipelined emission ----
        from collections import deque
        batches = [(r * BL + b, b) for r in range(reps) for b in range(BL)]
        nbat = len(batches)

        load_wqk()
        load_wv_qkb()
        emit_load_xb(0, 0)
        for mt in range(12):
            emit_qkproj(0, mt, 0)
            emit_qkproj(0, mt, 1)
        for tt in range(5):
            emit_vproj(0, tt, 0)
            emit_vproj(0, tt, 1)
        load_wp_expb()

        for idx, (i, b) in enumerate(batches):
            pre = deque()
            qkq = deque()
            if idx + 1 < nbat:
                i2, b2 = batches[idx + 1]
                pre.append(lambda i2=i2, b2=b2: emit_load_xb(i2, b2))
            if idx > 0:
                ip, bp = batches[idx - 1]
                for mt in range(6):
                    pre.append(lambda ip=ip, bp=bp, mt=mt:
                               (emit_proj(ip, bp, mt, 0),
                                emit_proj(ip, bp, mt, 1)))
            if idx + 1 < nbat:
                i2, b2 = batches[idx + 1]
                for tt in range(5):
                    pre.append(lambda i2=i2, tt=tt:
                               (emit_vproj(i2, tt, 0), emit_vproj(i2, tt, 1)))
                for mt in range(12):
                    qkq.append(lambda i2=i2, mt=mt:
                               (emit_qkproj(i2, mt, 0), emit_qkproj(i2, mt, 1)))
            for half in range(2):
                for hp in range(3):
                    he = half * 6 + 2 * hp
                    ho = he + 1
                    for jt in range(5):
                        emit_attn_qk(i, he, jt)
                        emit_attn_qk(i, ho, jt)
                        emit_attn_pv(i, he, jt)
                    if pre:
                        pre.popleft()()
                    for jt in range(5):
                        emit_attn_pv(i, ho, jt)
                    if qkq:
                        qkq.popleft()()
                        qkq.popleft()()
                emit_division(i, half)
            while pre:
                pre.popleft()()
            while qkq:
                qkq.popleft()()
        ilast, blast = batches[-1]
        for mt in range(6):
            emit_proj(ilast, blast, mt, 0)
            emit_proj(ilast, blast, mt, 1)

    nc.compile()
    _cache[key] = nc
    return nc


def _prep_inputs(x, qkv_w, q_bias, v_bias, rpb_table, rel_idx, proj_w, proj_b):
    x = np.asarray(x, np.float32)
    qkv_w = np.asarray(qkv_w, np.float32)
    q_bias = np.asarray(q_bias, np.float32)
    v_bias = np.asarray(v_bias, np.float32)
    rpb_table = np.asarray(rpb_table, np.float32)
    rel_idx = np.asarray(rel_idx)
    proj_w = np.asarray(proj_w, np.float32)
    proj_b = np.asarray(proj_b, np.float32)

    scale = D ** -0.5
    wqk_f = qkv_w[:2 * C].T.copy()            # [768, 1536]
    wqk_f[:, :C] *= scale
    wqk_np = wqk_f.astype(BF16)
    qb = np.concatenate([q_bias * scale, np.zeros(C, np.float32)])  # [1536]
    qkb_np = np.ascontiguousarray(qb.reshape(12, 128).T)            # [128, 12]
    wv_np = qkv_w[2 * C:].T.copy().astype(BF16)
    wp_np = proj_w.T.copy().astype(BF16)
    pb_eff = (proj_b + proj_w @ v_bias).astype(np.float32)          # [768]
    pb2_np = np.ascontiguousarray(pb_eff.reshape(6, 128).T)         # [128, 6]
    et = np.exp(rpb_table)                     # [2209, 12]
    idx = np.clip(np.asarray(rel_idx, np.int64), 0, et.shape[0] - 1)
    g = et[idx]                                # [576i, 576j, 12]
    expb_np = np.ascontiguousarray(g.transpose(2, 1, 0)).astype(BF16)

    in_maps = []
    for ci in range(NCORES):
        xc = x[ci * BL:(ci + 1) * BL]          # [8, 576, 768]
        xT_np = np.ascontiguousarray(
            xc.transpose(2, 0, 1).reshape(C, NTOK)).astype(BF16)
        in_maps.append({
            "xT": xT_np, "wqk": wqk_np, "qkb": qkb_np,
            "wv": wv_np, "wp": wp_np, "pb2": pb2_np, "expb": expb_np,
        })
    return in_maps


def kernel(x, qkv_w, q_bias, v_bias, rpb_table, rel_idx, proj_w, proj_b,
           _want_profile=False):
    in_maps = _prep_inputs(x, qkv_w, q_bias, v_bias, rpb_table, rel_idx,
                           proj_w, proj_b)
    nc = _build()
    from concourse.bass_utils import run_bass_kernel_spmd
    res = run_bass_kernel_spmd(nc, in_maps, core_ids=list(range(NCORES)),
                               trace=_want_profile)
    outs = [np.asarray(r["out"], np.float32).T.reshape(BL, N, C)
            for r in res.results]
    y = np.concatenate(outs, 0)
    if _want_profile:
        return y, res
    return y
